# revision 30
# baseline (speedup 1.0000x reference)
"""Self-contained Trainium2 attention-block kernel (8 NeuronCores, SPMD).

Problem: x[4,4096,128], Wq/Wk[64,128], Wv[128,128] ->
  softmax((x Wq^T)(x Wk^T)^T / 8) (x Wv^T)   -> [4,4096,128] f32

Sharding: data-parallel over batch (4) x query-halves (2) = 8 cores.
Each core: q rows 2048, full K (4096) via algebra below. No collectives.

Algebraic reformulation (projections folded into attention):
  scores[k,q] = x_k^T (Wk^T Wq) x_q = xT_chunk.T @ QW,
    QW = M^T x_q with M^T = Wq^T Wk precomputed on host  -> no K proj.
  out[v,q] = Wv^T R / D with R[d,q] = sum_k x[k,d] p[k,q] accumulated
    like PV over k chunks                                -> no V proj.
Both ST and R matmuls contract over the full 128 dim with lhsT taken
straight from DMA'd x layouts (xT [d,k] and x_kd [k, c*128+d]).

Per-core pipeline (all matmuls bf16), ACT-exp-bound steady state:
  ST[k,q] psum ring-3 x [128,1024] (6 banks) + R psum 2x1 bank = 8.
  exp on ScalarE with fused 1/8 scale (no max-subtract: scores ~N(0,1)).
  Softmax denominator: bf16 binary-counter tree on DVE over groups
  0..14; the last group's two chunks fold into the ones-matmul psum
  accumulation (dps), which shares a borrowed ST-ring slot with the
  Wv^T R output. Inputs stream on four DMA queues, weights first.
"""

import sys

sys.path.insert(0, "/opt/trn_rl_repo")

from contextlib import ExitStack

import ml_dtypes
import numpy as np

import concourse.bass as bass  # noqa: F401
import concourse.bacc as bacc
import concourse.tile as tile
from concourse import mybir
from concourse.bass_utils import run_bass_kernel_spmd

BF16 = mybir.dt.bfloat16
F32 = mybir.dt.float32
NPBF16 = ml_dtypes.bfloat16

B, S, D, A = 4, 4096, 128, 64
NQ = S // 2          # q rows per core
QB = 512             # q block (psum bank free size)
KC = 128             # k chunk (matmul contraction tile)
NKC = S // KC        # 32 chunks
NQB = NQ // QB       # 4 q blocks
GROUP = 2            # k chunks per exp group ([128,1024] psum tile)
NGRP = NKC // GROUP  # 16 groups per block
EXP = mybir.ActivationFunctionType.Exp

_CACHED_NC = None


def _log(msg):
    import time as _t
    print(f"[kernel {_t.strftime('%H:%M:%S')}] {msg}", file=sys.stderr, flush=True)


def build_nc():
    _log("build_nc: tracing graph")
    nc = bacc.Bacc(
        "TRN2", target_bir_lowering=False, debug=False,
        enable_asserts=False, num_devices=8,
    )
    xT = nc.dram_tensor("xT", [D, S], BF16, kind="ExternalInput").ap()
    xkd = nc.dram_tensor("xkd", [128, S], BF16, kind="ExternalInput").ap()
    xqT = nc.dram_tensor("xqT", [D, NQ], BF16, kind="ExternalInput").ap()
    # mT = Wq^T Wk | wvT = Wv^T | ones, packed: one small DMA, first
    wpack = nc.dram_tensor("wpack", [D, 384], BF16, kind="ExternalInput").ap()
    # outT layout [v, q]; host transposes for free during gather
    out = nc.dram_tensor("out", [D, NQ], BF16, kind="ExternalOutput").ap()

    with tile.TileContext(nc) as tc, ExitStack() as ctx:
        persist = ctx.enter_context(tc.tile_pool(name="persist", bufs=1))
        # PSUM: st ring 3x(2 banks) + R 2x(1 bank) = 8 banks.
        # dps/WvR and QW-proj tiles borrow ring slots via matching tags.
        ps_st = ctx.enter_context(tc.tile_pool(name="ps_st", bufs=3, space="PSUM"))
        ps_r = ctx.enter_context(tc.tile_pool(name="ps_r", bufs=2, space="PSUM"))
        ppool = ctx.enter_context(tc.tile_pool(name="ppool", bufs=7))
        tpool = ctx.enter_context(tc.tile_pool(name="tpool", bufs=10))
        mpool = ctx.enter_context(tc.tile_pool(name="mpool", bufs=2))

        # ---- persistent SBUF + input DMAs ----
        # Three queues share ~190GB/s; keep <=6 descriptors per queue
        # (the 7th recycles semaphores with wait-gating, serializing the
        # queue). Scalar leads with the qw0 critical path in minimal
        # slices; sync streams xT in consumption order; gpsimd takes xkd
        # (R-matmul lag is absorbed by the p-tile ring, so xkd can trail).
        # The first descriptor on each queue completes earliest, so the
        # three qw0/ST0 prerequisites (xqT0, mT, xT piece0) each lead a
        # queue; 11 descriptors total stays inside the semaphore pool.
        wp_s = persist.tile([D, 384], BF16, tag="wp_s")
        xqT_s = persist.tile([D, NQ], BF16, tag="xqT_s")
        xT_s = persist.tile([D, S], BF16, tag="xT_s")
        xkd_s = persist.tile([128, S], BF16, tag="xkd_s")
        # 1024-col pieces keep 2KB DMA lines (512-col slices halve DMA
        # throughput); wpack is tiny and leads the scalar queue
        nc.scalar.dma_start(wp_s[:], wpack[:])
        nc.sync.dma_start(xT_s[:, 0:1024], xT[:, 0:1024])
        nc.gpsimd.dma_start(xkd_s[:, 0:1024], xkd[:, 0:1024])
        nc.scalar.dma_start(xqT_s[:, 0:1024], xqT[:, 0:1024])
        nc.scalar.dma_start(xqT_s[:, 1024:], xqT[:, 1024:])
        warm = persist.tile([1, 1], F32, tag="warm")
        nc.gpsimd.memset(warm[:], 1.0)
        warm2 = persist.tile([1, 1], F32, tag="warm2")
        nc.scalar.activation(warm2[:], warm[:], EXP)

        for j in range(1, 4):
            nc.sync.dma_start(xT_s[:, j * 1024:(j + 1) * 1024],
                              xT[:, j * 1024:(j + 1) * 1024])
        nc.gpsimd.dma_start(xkd_s[:, 1024:2048], xkd[:, 1024:2048])
        nc.gpsimd.dma_start(xkd_s[:, 2048:4096], xkd[:, 2048:4096])
        mT_s = wp_s[:, 0:128]     # [d2, d] = Wq^T Wk
        wv_s = wp_s[:, 128:256]   # [d, v] = Wv^T
        ones_s = wp_s[:, 256:384]

        QW_s = persist.tile([128, NQ], BF16, tag="QW_s")  # [d, q]

        # ---- QW projection: QW[:, jQB:(j+1)QB] = mT.T @ xqT block ----
        def qw_mm(j):
            pt = ps_st.tile([128, GROUP * QB], F32, tag="st", name=f"qw{j}")
            nc.tensor.matmul(pt[:, 0:QB], mT_s,
                             xqT_s[:, j * QB:(j + 1) * QB],
                             start=True, stop=True)
            return pt

        def qw_copy(j, pt):
            nc.vector.tensor_copy(QW_s[:, j * QB:(j + 1) * QB], pt[:, 0:QB])

        pt0 = qw_mm(0)
        nc.vector.tensor_copy(QW_s[:, 0:QB], pt0[:, 0:QB])  # critical path

        # ---- attention: flat software pipeline over (qblock, group) ----
        ALL = [(qb, g) for qb in range(NQB) for g in range(NGRP)]

        def emit_st(qb, g):
            q0 = qb * QB
            st = ps_st.tile([128, GROUP * QB], F32, tag="st")
            for i in range(GROUP):
                kc = g * GROUP + i
                nc.tensor.matmul(st[:, i * QB:(i + 1) * QB],
                                 xT_s[:, kc * KC:(kc + 1) * KC],
                                 QW_s[:, q0:q0 + QB],
                                 start=True, stop=True)
            return st

        st_tiles = {}
        for k in range(3):
            st_tiles[ALL[k]] = emit_st(*ALL[k])

        r_tiles = {}
        accs = {}       # qb -> running bf16 chunk-sum [128, QB]
        leaf_last = {}  # qb -> the g14 leaf, folded into dps directly

        # dribble the remaining QW blocks into later groups (QW j feeds
        # ST group 16j, emitted at idx 16j-3), well behind the xqT DMA
        qw_pend = {}
        QW_MM_AT = {8: 1, 20: 2, 36: 3}
        QW_CP_AT = {10: 1, 22: 2, 38: 3}

        for idx, (qb, g) in enumerate(ALL):
            st = st_tiles.pop((qb, g))
            p = ppool.tile([128, GROUP * QB], BF16, tag="p")
            nc.scalar.activation(p[:], st[:], EXP, scale=0.125)
            if qb not in r_tiles:
                r_tiles[qb] = ps_r.tile([128, QB], F32, tag="pv",
                                        name=f"r{qb}")
            rt = r_tiles[qb]
            for i in range(GROUP):
                kc = g * GROUP + i
                nc.tensor.matmul(rt[:], xkd_s[:, kc * KC:(kc + 1) * KC],
                                 p[:, i * QB:(i + 1) * QB],
                                 start=(kc == 0), stop=(kc == NKC - 1))
            if idx + 3 < len(ALL):
                st_tiles[ALL[idx + 3]] = emit_st(*ALL[idx + 3])
            if idx in QW_MM_AT:
                j = QW_MM_AT[idx]
                qw_pend[j] = qw_mm(j)
            if idx in QW_CP_AT:
                j = QW_CP_AT[idx]
                qw_copy(j, qw_pend.pop(j))

            if g < NGRP - 1:
                # leaf: sum the group's two chunks (every 3rd on GpSimd),
                # then fold into the running bf16 accumulator on DVE
                t1 = tpool.tile([128, QB], BF16, tag="tr")
                # GpSimd leaves only early in the qblock: late ones sit on
                # the D chain that gates the dps matmul and the PE queue
                leaf_eng = nc.gpsimd if (g % 3 == 2 and g < 12) \
                    else nc.vector
                leaf_eng.tensor_add(t1[:], p[:, 0:QB], p[:, QB:2 * QB])
                if g == NGRP - 2:
                    # keep the g14 leaf out of the running sum: it joins
                    # the dps matmul accumulation directly, so no serial
                    # DVE add sits on the qblock tail
                    leaf_last[qb] = t1
                elif qb not in accs:
                    accs[qb] = t1
                else:
                    na = tpool.tile([128, QB], BF16, tag="tr")
                    nc.vector.tensor_add(na[:], accs[qb][:], t1[:])
                    accs[qb] = na
            else:
                # last group: fold the g14 leaf and this group's two chunks
                # into the ones-matmul psum accumulation -> D arrives
                # broadcast across partitions. dps shares a borrowed
                # st-ring tile with the Wv^T R output.
                s_tile = accs.pop(qb)
                fin = ps_st.tile([128, GROUP * QB], F32, tag="st",
                                 name=f"fin{qb}")
                nc.tensor.matmul(fin[:, QB:2 * QB], ones_s, s_tile[:],
                                 start=True, stop=False)
                nc.tensor.matmul(fin[:, QB:2 * QB], ones_s,
                                 leaf_last.pop(qb)[:],
                                 start=False, stop=False)
                nc.tensor.matmul(fin[:, QB:2 * QB], ones_s, p[:, 0:QB],
                                 start=False, stop=False)
                nc.tensor.matmul(fin[:, QB:2 * QB], ones_s, p[:, QB:2 * QB],
                                 start=False, stop=True)
                rb = mpool.tile([128, QB], BF16, tag="rb")
                dinvb = mpool.tile([128, QB], F32, tag="dinvb")
                outf = mpool.tile([128, QB], BF16, tag="outf")
                HB = QB // 2
                if qb < NQB - 1:
                    nc.vector.tensor_copy(rb[:], rt[:])  # frees R psum bank
                    nc.tensor.matmul(fin[:, 0:QB], wv_s, rb[:],
                                     start=True, stop=True)
                    nc.vector.reciprocal_approx_fast(dinvb[:],
                                                     fin[:, QB:2 * QB])
                    nc.vector.tensor_mul(outf[:], fin[:, 0:QB], dinvb[:])
                    nc.gpsimd.dma_start(out[:, qb * QB:(qb + 1) * QB],
                                        outf[:])
                else:
                    # final qblock: half-split casts/WvR overlap the R-stop,
                    # then one full-width recip+mul+DMA
                    nc.vector.tensor_copy(rb[:, 0:HB], rt[:, 0:HB])
                    nc.vector.tensor_copy(rb[:, HB:QB], rt[:, HB:QB])
                    nc.tensor.matmul(fin[:, 0:HB], wv_s, rb[:, 0:HB],
                                     start=True, stop=True)
                    nc.tensor.matmul(fin[:, HB:QB], wv_s, rb[:, HB:QB],
                                     start=True, stop=True)
                    nc.vector.reciprocal_approx_fast(dinvb[:],
                                                     fin[:, QB:2 * QB])
                    nc.vector.tensor_mul(outf[:], fin[:, 0:QB], dinvb[:])
                    nc.sync.dma_start(out[:, qb * QB:(qb + 1) * QB],
                                      outf[:])
                del r_tiles[qb]

    _log("build_nc: bacc compile")
    nc.compile()
    _log("build_nc: done")
    return nc


def _host_prep(x, Wq, Wk, Wv):
    x = np.asarray(x, dtype=np.float32)
    Wq = np.asarray(Wq, dtype=np.float32)
    Wk = np.asarray(Wk, dtype=np.float32)
    Wv = np.asarray(Wv, dtype=np.float32)
    mT = Wq.T @ Wk                      # [d2, d]
    wpack = np.concatenate(
        [mT, Wv.T, np.ones((D, D), np.float32)], axis=1).astype(NPBF16)
    wpack = np.ascontiguousarray(wpack)
    in_maps = []
    for c in range(8):
        b, h = c // 2, c % 2
        xb = x[b]                       # [S, D]
        xkd = np.ascontiguousarray(
            xb.reshape(NKC, KC, D).transpose(1, 0, 2).reshape(KC, NKC * D)
        ).astype(NPBF16)                # [k, c*128+d]
        in_maps.append({
            "xT": np.ascontiguousarray(xb.T).astype(NPBF16),
            "xkd": xkd,
            "xqT": np.ascontiguousarray(
                xb[h * NQ:(h + 1) * NQ].T).astype(NPBF16),
            "wpack": wpack,
        })
    return in_maps


def run(x, Wq, Wk, Wv, trace=False, **kw):
    global _CACHED_NC
    if _CACHED_NC is None:
        _CACHED_NC = build_nc()
    in_maps = _host_prep(x, Wq, Wk, Wv)
    _log("run_bass_kernel_spmd (includes NEFF compile on first call)")
    res = run_bass_kernel_spmd(
        _CACHED_NC, in_maps, core_ids=list(range(8)), trace=trace, **kw)
    _log("run_bass_kernel_spmd returned")
    full = np.zeros((B, S, D), np.float32)
    for c in range(8):
        b, h = c // 2, c % 2
        full[b, h * NQ:(h + 1) * NQ] = np.asarray(
            res.results[c]["out"]).astype(np.float32).T
    return full, res


def kernel(x, Wq, Wk, Wv):
    full, _ = run(x, Wq, Wk, Wv, trace=False)
    return full
